# revision 22
# baseline (speedup 1.0000x reference)
"""Block-sparse (view-causal) multi-head attention on 8 TRN2 NeuronCores.

Full inputs in, full output out. Sharding: data-parallel over batch (B=2),
tensor-parallel over heads (16 heads -> 4 per core). Each core computes its
4 heads' attention + its slice of the output projection; the host sums the
4 head-group partial projections per batch (the tensor-parallel reduce).

Device-side pipeline (single fused emission stream, phases interleaved so
the Scalar engine's EXP work overlaps PE work instead of serializing):
  - inputs DMA'd in 512-token panels so projections start early
  - q/k kept transposed (qT/kT [dh, tokens]) so score matmuls need no
    transposes; V is augmented with a ones column so each PV matmul
    accumulates both sum(exp*v) and the softmax denominator in PSUM
  - attention emitted per query view; "filler" PE work (later panels'
    projections, output projections) is interleaved between the
    EXP-dependent attention ops so the PE FIFO never stalls on Scalar
  - per-view output projection + DMA so the 4MB output streams out
    during the attention phase instead of after it
All matmul operands are bf16 (PSUM accumulation in fp32).
"""

import sys
from collections import deque

if "/opt/trn_rl_repo" not in sys.path:
    sys.path.insert(0, "/opt/trn_rl_repo")

import numpy as np
import ml_dtypes

B, V, L, C, H = 2, 8, 256, 1024, 16
S = V * L                # 2048 tokens
DH = C // H              # 64
HPC = 4                  # heads per core
CPB = HPC * DH           # 256 channel block per core
N_CORES = 8
SCALE = DH ** -0.5       # 1/8, folded into the exp activation
KC = C // 128            # 8 contraction chunks for the projections
NS = 4                   # 512-token panels
SC = S // 128            # 16 sequence chunks

_compiled = {}
LAST_RESULTS = None
# legacy flags kept for test-harness compatibility
PACK_QK = True
SAFE_RECIP = False
SPLIT_ACT = False


def _allowed(qv):
    """View-level mask row: views 0/1 cross-attend only; views >=2 block-causal."""
    if qv == 0:
        return [1]
    if qv == 1:
        return [0]
    return list(range(qv + 1))


def build():
    import concourse.tile as tile
    from concourse import bacc, mybir
    from concourse.masks import make_identity

    f32 = mybir.dt.float32
    bf16 = mybir.dt.bfloat16
    f8 = mybir.dt.float8e4
    EXP = mybir.ActivationFunctionType.Exp

    nc = bacc.Bacc("TRN2", target_bir_lowering=False, debug=False,
                   num_devices=N_CORES)
    xP = nc.dram_tensor("xP", [128, KC * S], bf16, kind="ExternalInput").ap()
    wqP = nc.dram_tensor("wqP", [128, KC * CPB], bf16, kind="ExternalInput").ap()
    wkP = nc.dram_tensor("wkP", [128, KC * CPB], bf16, kind="ExternalInput").ap()
    wvP = nc.dram_tensor("wvP", [128, KC * CPB], bf16, kind="ExternalInput").ap()
    wpP = nc.dram_tensor("wpP", [128, 2 * C], bf16, kind="ExternalInput").ap()
    y = nc.dram_tensor("y", [S, C], bf16, kind="ExternalOutput").ap()

    HW = KC * CPB // 2   # half-weight free size

    with tile.TileContext(nc) as tc:
        with (
            tc.tile_pool(name="wts", bufs=1) as w_pool,
            tc.tile_pool(name="xt", bufs=1) as xt_pool,
            tc.tile_pool(name="qk", bufs=1) as qk_pool,
            tc.tile_pool(name="va", bufs=1) as va_pool,
            tc.tile_pool(name="ot", bufs=1) as ot_pool,
            tc.tile_pool(name="on", bufs=1) as on_pool,
            tc.tile_pool(name="exp", bufs=1) as exp_pool,
            tc.tile_pool(name="small", bufs=1) as small_pool,
            tc.tile_pool(name="ysb", bufs=3) as ysb_pool,
            tc.tile_pool(name="psp", bufs=2, space="PSUM") as psum_p,
            tc.tile_pool(name="pss", bufs=2, space="PSUM") as psum_s,
            tc.tile_pool(name="pso", bufs=2, space="PSUM") as psum_o,
        ):
            # ---- weights (in halves for earlier readiness) + x panels ----
            wh = {}
            for nm, dram in (("wq", wqP), ("wk", wkP), ("wv", wvP)):
                for half in range(2):
                    t = w_pool.tile([128, HW], bf16, tag=f"{nm}{half}")
                    wh[(nm, half)] = t
            wp_sb = w_pool.tile([128, 2 * C], bf16, tag="wp")
            def wslice(nm, k, lo, hi):
                # columns [lo, hi) of contraction-chunk k of weight nm
                half, kk = divmod(k, KC // 2)
                return wh[(nm, half)][:, kk * CPB + lo:kk * CPB + hi]

            # trn2 has exactly two HW-DGE rings (sync + scalar), each FIFO:
            # split the startup-critical transfers across both so panel 0
            # and wq/wk arrive as early as possible.
            xts = {}  # (k, n) -> (tile, col offset)

            def xsl(k, n, lo=0, hi=512):
                t, base = xts[(k, n)]
                return t[:, base + lo:base + hi]

            def dma_x(k, n, eng, width=512):
                t = xt_pool.tile([128, width], bf16, tag=f"xt{width}",
                                 bufs=(16 if width == 512 else 8),
                                 name=f"xt{k}_{n}")
                eng.dma_start(
                    t[:], xP[:, (k * NS + n) * 512:(k * NS + n) * 512 + width])
                for w in range(width // 512):
                    xts[(k, n + w)] = (t, w * 512)

            # ring A (sync) / ring B (scalar), interleaved by arrival need
            nc.scalar.dma_start(wh[("wq", 0)][:], wqP[:, 0:HW])
            for k in range(4):
                dma_x(k, 0, nc.sync)
            nc.scalar.dma_start(wh[("wq", 1)][:], wqP[:, HW:2 * HW])
            for k in range(4, KC):
                dma_x(k, 0, nc.scalar)
            nc.sync.dma_start(wh[("wk", 0)][:], wkP[:, 0:HW])
            nc.sync.dma_start(wh[("wk", 1)][:], wkP[:, HW:2 * HW])
            nc.scalar.dma_start(wh[("wv", 0)][:], wvP[:, 0:HW])
            nc.scalar.dma_start(wh[("wv", 1)][:], wvP[:, HW:2 * HW])
            for k in range(KC):  # panel 1 split across both rings
                dma_x(k, 1, nc.sync if k % 2 == 0 else nc.scalar)
            nc.scalar.dma_start(wp_sb[:], wpP[:, :])
            for k in range(KC):  # panels 2+3 merged into 256KB transfers
                dma_x(k, 2, nc.sync if k % 2 == 0 else nc.scalar, width=1024)

            # ---- constants + PE clock warmup while input DMAs run ----
            onesc = small_pool.tile([128, HPC], bf16, tag="onesc")
            nc.vector.memset(onesc[:], 1.0)
            junkw = small_pool.tile([128, 128], bf16, tag="junkw")
            nc.vector.memset(junkw[:], 0.5)
            junkm = small_pool.tile([128, 256], bf16, tag="junkm")
            nc.vector.memset(junkm[:], 0.5)
            for _ in range(6):
                wps = psum_p.tile([128, 512], f32, tag="psp", name="warm")
                nc.tensor.matmul(wps[:, 0:256], junkw[:], junkm[:],
                                 start=True, stop=True)
            # identity for the PE-mode transposes; first needed mid-kernel
            ident = small_pool.tile([128, 128], bf16, tag="ident")
            make_identity(nc, ident[:])

            # ---- persistent activation tiles ----
            qk_tiles = {}
            for p in ("q", "k"):
                for m in range(2):
                    qk_tiles[(p, m)] = qk_pool.tile(
                        [128, S], bf16, tag=f"{p}T{m}", name=f"{p}T{m}")
            va = [va_pool.tile([128, HPC * 65], bf16, tag="va", bufs=SC,
                               name=f"va{sc}") for sc in range(SC)]
            on_tiles = [on_pool.tile([128, CPB], bf16, tag="on", bufs=SC,
                                     name=f"on{sc}") for sc in range(SC)]
            ot_tiles = [ot_pool.tile([128, S], bf16, tag=f"ot{m}",
                                     name=f"oT{m}") for m in range(2)]

            # ---- emitters ----
            def emit_qk_proj_group(p, m, n):
                nm = "wq" if p == "q" else "wk"
                dst = qk_tiles[(p, m)]
                ps = psum_p.tile([128, 512], f32, tag="psp", name="psproj")
                for k in range(KC):
                    nc.tensor.matmul(
                        ps[:],
                        wslice(nm, k, m * 128, (m + 1) * 128),
                        xsl(k, n),
                        start=(k == 0), stop=(k == KC - 1))
                nc.vector.tensor_copy(dst[:, n * 512:(n + 1) * 512], ps[:])

            def emit_va_group(sc):
                t = va[sc]
                n, r = divmod(sc, 4)
                tones = t[:].rearrange("p (h x) -> p h x", x=65)[:, :, 64:65]
                nc.vector.tensor_copy(
                    tones, onesc[:].rearrange("p (h x) -> p h x", x=1))
                ps = psum_p.tile([128, CPB], f32, tag="psp", name="psv")
                for k in range(KC):
                    nc.tensor.matmul(
                        ps[:],
                        xsl(k, n, r * 128, (r + 1) * 128),
                        wslice("wv", k, 0, CPB),
                        start=(k == 0), stop=(k == KC - 1))
                tv = t[:].rearrange("p (h x) -> p h x", x=65)[:, :, 0:64]
                pv = ps[:].rearrange("p (h d) -> p h d", d=64)
                nc.vector.tensor_copy(tv, pv)

            def emit_transpose(sc):
                for half in range(2):
                    pt = psum_p.tile([128, 128], bf16, tag="psp", name="pt")
                    nc.tensor.transpose(
                        pt[:], on_tiles[sc][:, half * 128:(half + 1) * 128],
                        ident[:])
                    nc.vector.tensor_copy(
                        ot_tiles[half][:, sc * 128:(sc + 1) * 128], pt[:])

            def emit_y_proj(sc):
                ys = ysb_pool.tile([128, C], bf16, tag="ysb")
                pss2 = [psum_p.tile([128, 512], f32, tag="psp", name="psy")
                        for _ in range(2)]
                for m in range(2):  # m outer: reuse stationary across n2
                    for n2 in range(2):
                        nc.tensor.matmul(
                            pss2[n2][:],
                            ot_tiles[m][:, sc * 128:(sc + 1) * 128],
                            wp_sb[:, m * C + n2 * 512:m * C + (n2 + 1) * 512],
                            start=(m == 0), stop=(m == 1))
                nc.vector.tensor_copy(ys[:, 0:512], pss2[0][:])
                nc.vector.tensor_copy(ys[:, 512:1024], pss2[1][:])
                nc.sync.dma_start(y[sc * 128:(sc + 1) * 128, :], ys[:])

            # filler queue: PE work emitted between EXP-dependent attention
            # ops so the PE FIFO never stalls waiting on the Scalar engine.
            # Entries carry a (kind, key) tag so dependency deadlines can
            # force-drain them before the attention work that consumes them.
            fillers = deque()  # (kind, key, avail_qv, emit_fn)

            def take_filler(cur_qv, n=1):
                # pop the first filler whose availability window has opened
                # (tr/y fillers are held back so the late, filler-hungry
                # views still have PE work to interleave)
                for _ in range(n):
                    for idx, (kind, key, avail, fn) in enumerate(fillers):
                        if avail <= cur_qv:
                            del fillers[idx]
                            fn()
                            break
                    else:
                        return

            def drain_fillers(pred):
                keep = deque()
                while fillers:
                    kind, key, avail, fn = fillers.popleft()
                    if pred(kind, key):
                        fn()
                    else:
                        keep.append((kind, key, avail, fn))
                fillers.extend(keep)

            def push_panel_fillers(n):
                for m in range(2):
                    for p in ("q", "k"):
                        fillers.append(
                            ("qk", n, 0,
                             lambda p=p, m=m, n=n: emit_qk_proj_group(p, m, n)))
                for sc in range(4 * n, 4 * n + 4):
                    fillers.append(
                        ("va", sc, 0, lambda sc=sc: emit_va_group(sc)))

            def emit_qk_phase(m, qv):
                kT = qk_tiles[("k", m)]
                qT = qk_tiles[("q", m)]
                qs = slice(qv * 256, (qv + 1) * 256)
                kvs = _allowed(qv)
                ets = []
                for i, kv in enumerate(kvs):
                    pss = psum_s.tile([128, 1024], f32, tag="pss", name="pss")
                    for j in range(2):
                        kc = 2 * kv + j
                        for h in range(2):
                            nc.tensor.matmul(
                                pss[:, (2 * h + j) * 256:
                                    (2 * h + j + 1) * 256],
                                kT[64 * h:64 * (h + 1),
                                   kc * 128:(kc + 1) * 128],
                                qT[64 * h:64 * (h + 1), qs],
                                start=True, stop=True)
                    et = exp_pool.tile([128, 1024], bf16, tag="exp", bufs=20)
                    nc.scalar.activation(et[:], pss[:], EXP,
                                         scale=float(SCALE))
                    ets.append((kv, et))
                    if i >= 1:
                        take_filler(qv, 1)
                if len(kvs) <= 3:
                    take_filler(qv, 1)
                return ets

            def emit_pv_phase(m, qv, ets, qcs=(0, 1)):
                # PV accumulation groups per (h, q-chunk); the ones column of
                # va lands the softmax denominator in PSUM column 64
                last = len(ets) - 1
                rp = small_pool.tile([128, 4], f32, tag="rp", bufs=4,
                                     name="rp")
                for h in range(2):
                    hh = 2 * m + h
                    for qc in qcs:
                        g = 2 * h + qc
                        pg = psum_o.tile([128, 65], f32, tag="pso",
                                         name=f"pg{g}")
                        for i, (kv, et) in enumerate(ets):
                            for j in range(2):
                                kc = 2 * kv + j
                                nc.tensor.matmul(
                                    pg[:],
                                    et[:, (2 * h + j) * 256 + qc * 128:
                                       (2 * h + j) * 256 + qc * 128 + 128],
                                    va[kc][:, hh * 65:(hh + 1) * 65],
                                    start=(i == 0 and j == 0),
                                    stop=(i == last and j == 1))
                        sc = qv * 2 + qc
                        nc.vector.reciprocal(rp[:, g:g + 1], pg[:, 64:65])
                        nc.vector.tensor_scalar_mul(
                            on_tiles[sc][:, hh * 64:(hh + 1) * 64],
                            pg[:, 0:64],
                            rp[:, g:g + 1])

            # ---- main emission: panel 0 directly, then attention with
            # later panels' projections + transposes/y-projs as PE fillers.
            # Per view: both head-pairs' QK phases back-to-back (m1's QK
            # covers m0's EXP latency), then both PV phases. ----
            for m in range(2):
                for p in ("q", "k"):
                    emit_qk_proj_group(p, m, 0)
            for sc in range(4):
                emit_va_group(sc)
            push_panel_fillers(1)
            for n in range(NS):
                for qv in (2 * n, 2 * n + 1):
                    # hard deadlines: this view's q/k panels + va chunks
                    drain_fillers(lambda kind, key, qv=qv:
                                  (kind == "qk" and key <= qv // 2) or
                                  (kind == "va" and key <= 2 * qv + 1))
                    ets0 = emit_qk_phase(0, qv)
                    ets1 = emit_qk_phase(1, qv)
                    if qv == V - 1:
                        # final view: finish each q-chunk end-to-end so the
                        # kernel tail is one transpose + y-proj, not four
                        for qc in range(2):
                            emit_pv_phase(0, qv, ets0, (qc,))
                            take_filler(qv, 1)
                            emit_pv_phase(1, qv, ets1, (qc,))
                            emit_transpose(2 * qv + qc)
                            emit_y_proj(2 * qv + qc)
                        continue
                    emit_pv_phase(0, qv, ets0)
                    take_filler(qv, 1)
                    emit_pv_phase(1, qv, ets1)
                    for sc in (2 * qv, 2 * qv + 1):
                        avail = min(7, sc // 2 + 3)
                        fillers.append(
                            ("tr", sc, avail, lambda sc=sc: emit_transpose(sc)))
                        fillers.append(
                            ("y", sc, avail, lambda sc=sc: emit_y_proj(sc)))
                if n + 2 <= NS - 1:
                    push_panel_fillers(n + 2)
            import os
            if os.environ.get("DBG_FILLERS"):
                sys.stderr.write(
                    f"leftover fillers at tail: {[(k, key) for k, key, _, _ in fillers]}\n")
            while fillers:
                fillers.popleft()[3]()

    nc.compile()
    return nc


def _get_compiled():
    if "nc" not in _compiled:
        _compiled["nc"] = build()
    return _compiled["nc"]


def make_in_maps(x, Wq, Wk, Wv, Wp):
    bf = ml_dtypes.bfloat16
    xf = np.asarray(x, np.float32).reshape(B, S, C)

    def pack_w(wT):  # [C, CPB] -> [128, KC*CPB]
        return np.ascontiguousarray(
            wT.reshape(KC, 128, CPB).transpose(1, 0, 2).reshape(128, KC * CPB)
        ).astype(bf)

    in_maps = []
    for c in range(N_CORES):
        b, g = divmod(c, HPC)
        hs = slice(g * CPB, (g + 1) * CPB)
        xT = xf[b].T  # [C, S]
        xPm = np.ascontiguousarray(
            xT.reshape(KC, 128, NS, 512)
            .transpose(1, 0, 2, 3).reshape(128, KC * S)).astype(bf)
        wpT = np.asarray(Wp, np.float32)[:, hs].T  # [CPB, C]
        wpm = np.ascontiguousarray(
            wpT.reshape(2, 128, C).transpose(1, 0, 2).reshape(128, 2 * C)
        ).astype(bf)
        in_maps.append({
            "xP": xPm,
            "wqP": pack_w(np.asarray(Wq, np.float32)[hs].T),
            "wkP": pack_w(np.asarray(Wk, np.float32)[hs].T),
            "wvP": pack_w(np.asarray(Wv, np.float32)[hs].T),
            "wpP": wpm,
        })
    return in_maps


def kernel(x, Wq, Wk, Wv, Wp, bp, _trace=False, _tmpdir=None):
    global LAST_RESULTS
    from concourse import bass_utils

    nc = _get_compiled()
    in_maps = make_in_maps(x, Wq, Wk, Wv, Wp)
    kwargs = {}
    if _trace:
        kwargs = {"trace": True, "tmpdir": _tmpdir}
    res = bass_utils.run_bass_kernel_spmd(
        nc, in_maps, core_ids=list(range(N_CORES)), **kwargs)
    LAST_RESULTS = res
    yout = np.zeros((B, S, C), np.float32)
    for c in range(N_CORES):
        yout[c // HPC] += res.results[c]["y"].astype(np.float32)
    yout += np.asarray(bp, np.float32).reshape(1, 1, C)
    return yout.reshape(B, V, L, C)


# revision 23
# speedup vs baseline: 1.0202x; 1.0202x over previous
"""Block-sparse (view-causal) multi-head attention on 8 TRN2 NeuronCores.

Full inputs in, full output out. Sharding: data-parallel over batch (B=2),
tensor-parallel over heads (16 heads -> 4 per core). Each core computes its
4 heads' attention + its slice of the output projection; the host sums the
4 head-group partial projections per batch (the tensor-parallel reduce).

Device-side pipeline (single fused emission stream, phases interleaved so
the Scalar engine's EXP work overlaps PE work instead of serializing):
  - inputs DMA'd in 512-token panels so projections start early
  - q/k kept transposed (qT/kT [dh, tokens]) so score matmuls need no
    transposes; V is augmented with a ones column so each PV matmul
    accumulates both sum(exp*v) and the softmax denominator in PSUM
  - attention emitted per query view; "filler" PE work (later panels'
    projections, output projections) is interleaved between the
    EXP-dependent attention ops so the PE FIFO never stalls on Scalar
  - per-view output projection + DMA so the 4MB output streams out
    during the attention phase instead of after it
All matmul operands are bf16 (PSUM accumulation in fp32).
"""

import sys
from collections import deque

if "/opt/trn_rl_repo" not in sys.path:
    sys.path.insert(0, "/opt/trn_rl_repo")

import numpy as np
import ml_dtypes

B, V, L, C, H = 2, 8, 256, 1024, 16
S = V * L                # 2048 tokens
DH = C // H              # 64
HPC = 4                  # heads per core
CPB = HPC * DH           # 256 channel block per core
N_CORES = 8
SCALE = DH ** -0.5       # 1/8, folded into the exp activation
KC = C // 128            # 8 contraction chunks for the projections
NS = 4                   # 512-token panels
SC = S // 128            # 16 sequence chunks

_compiled = {}
LAST_RESULTS = None
# legacy flags kept for test-harness compatibility
PACK_QK = True
SAFE_RECIP = False
SPLIT_ACT = False


def _allowed(qv):
    """View-level mask row: views 0/1 cross-attend only; views >=2 block-causal."""
    if qv == 0:
        return [1]
    if qv == 1:
        return [0]
    return list(range(qv + 1))


def build():
    import concourse.tile as tile
    from concourse import bacc, mybir
    from concourse.masks import make_identity

    f32 = mybir.dt.float32
    bf16 = mybir.dt.bfloat16
    f8 = mybir.dt.float8e4
    EXP = mybir.ActivationFunctionType.Exp

    nc = bacc.Bacc("TRN2", target_bir_lowering=False, debug=False,
                   num_devices=N_CORES)
    xP = nc.dram_tensor("xP", [128, KC * S], bf16, kind="ExternalInput").ap()
    wqP = nc.dram_tensor("wqP", [128, KC * CPB], bf16, kind="ExternalInput").ap()
    wkP = nc.dram_tensor("wkP", [128, KC * CPB], bf16, kind="ExternalInput").ap()
    wvP = nc.dram_tensor("wvP", [128, KC * CPB], bf16, kind="ExternalInput").ap()
    wpP = nc.dram_tensor("wpP", [128, 2 * C], bf16, kind="ExternalInput").ap()
    y = nc.dram_tensor("y", [S, C], bf16, kind="ExternalOutput").ap()

    HW = KC * CPB // 2   # half-weight free size

    with tile.TileContext(nc) as tc:
        with (
            tc.tile_pool(name="wts", bufs=1) as w_pool,
            tc.tile_pool(name="xt", bufs=1) as xt_pool,
            tc.tile_pool(name="qk", bufs=1) as qk_pool,
            tc.tile_pool(name="va", bufs=1) as va_pool,
            tc.tile_pool(name="ot", bufs=1) as ot_pool,
            tc.tile_pool(name="on", bufs=1) as on_pool,
            tc.tile_pool(name="exp", bufs=1) as exp_pool,
            tc.tile_pool(name="small", bufs=1) as small_pool,
            tc.tile_pool(name="ysb", bufs=3) as ysb_pool,
            tc.tile_pool(name="psp", bufs=2, space="PSUM") as psum_p,
            tc.tile_pool(name="pss", bufs=2, space="PSUM") as psum_s,
            tc.tile_pool(name="pso", bufs=2, space="PSUM") as psum_o,
        ):
            # ---- weights (in halves for earlier readiness) + x panels ----
            wh = {}
            for nm, dram in (("wq", wqP), ("wk", wkP), ("wv", wvP)):
                for half in range(2):
                    t = w_pool.tile([128, HW], bf16, tag=f"{nm}{half}")
                    wh[(nm, half)] = t
            wp_sb = w_pool.tile([128, 2 * C], bf16, tag="wp")
            def wslice(nm, k, lo, hi):
                # columns [lo, hi) of contraction-chunk k of weight nm
                half, kk = divmod(k, KC // 2)
                return wh[(nm, half)][:, kk * CPB + lo:kk * CPB + hi]

            # trn2 has exactly two HW-DGE rings (sync + scalar), each FIFO:
            # split the startup-critical transfers across both so panel 0
            # and wq/wk arrive as early as possible.
            xts = {}  # (k, n) -> (tile, col offset)

            def xsl(k, n, lo=0, hi=512):
                t, base = xts[(k, n)]
                return t[:, base + lo:base + hi]

            def dma_x(k, n, eng, width=512):
                t = xt_pool.tile([128, width], bf16, tag=f"xt{width}",
                                 bufs=(16 if width == 512 else 8),
                                 name=f"xt{k}_{n}")
                eng.dma_start(
                    t[:], xP[:, (k * NS + n) * 512:(k * NS + n) * 512 + width])
                for w in range(width // 512):
                    xts[(k, n + w)] = (t, w * 512)

            # ring A (sync) / ring B (scalar), interleaved by arrival need
            nc.scalar.dma_start(wh[("wq", 0)][:], wqP[:, 0:HW])
            for k in range(4):
                dma_x(k, 0, nc.sync)
            nc.scalar.dma_start(wh[("wq", 1)][:], wqP[:, HW:2 * HW])
            for k in range(4, KC):
                dma_x(k, 0, nc.scalar)
            nc.sync.dma_start(wh[("wk", 0)][:], wkP[:, 0:HW])
            nc.sync.dma_start(wh[("wk", 1)][:], wkP[:, HW:2 * HW])
            nc.scalar.dma_start(wh[("wv", 0)][:], wvP[:, 0:HW])
            nc.scalar.dma_start(wh[("wv", 1)][:], wvP[:, HW:2 * HW])
            for k in range(KC):  # panel 1 split across both rings
                dma_x(k, 1, nc.sync if k % 2 == 0 else nc.scalar)
            nc.scalar.dma_start(wp_sb[:], wpP[:, :])
            for k in range(KC):  # panels 2+3 merged into 256KB transfers
                dma_x(k, 2, nc.sync if k % 2 == 0 else nc.scalar, width=1024)

            # ---- constants + PE clock warmup while input DMAs run ----
            onesc = small_pool.tile([128, HPC], bf16, tag="onesc")
            nc.vector.memset(onesc[:], 1.0)
            junkw = small_pool.tile([128, 128], bf16, tag="junkw")
            nc.vector.memset(junkw[:], 0.5)
            junkm = small_pool.tile([128, 256], bf16, tag="junkm")
            nc.vector.memset(junkm[:], 0.5)
            for _ in range(6):
                wps = psum_p.tile([128, 512], f32, tag="psp", name="warm")
                nc.tensor.matmul(wps[:, 0:256], junkw[:], junkm[:],
                                 start=True, stop=True)
            # identity for the PE-mode transposes; first needed mid-kernel
            ident = small_pool.tile([128, 128], bf16, tag="ident")
            make_identity(nc, ident[:])

            # ---- persistent activation tiles ----
            qk_tiles = {}
            for p in ("q", "k"):
                for m in range(2):
                    qk_tiles[(p, m)] = qk_pool.tile(
                        [128, S], bf16, tag=f"{p}T{m}", name=f"{p}T{m}")
            va = [va_pool.tile([128, HPC * 65], bf16, tag="va", bufs=SC,
                               name=f"va{sc}") for sc in range(SC)]
            on_tiles = [on_pool.tile([128, CPB], bf16, tag="on", bufs=SC,
                                     name=f"on{sc}") for sc in range(SC)]
            ot_tiles = [ot_pool.tile([128, S], bf16, tag=f"ot{m}",
                                     name=f"oT{m}") for m in range(2)]

            # ---- emitters ----
            def emit_qk_proj_group(p, m, n):
                nm = "wq" if p == "q" else "wk"
                dst = qk_tiles[(p, m)]
                ps = psum_p.tile([128, 512], f32, tag="psp", name="psproj")
                for k in range(KC):
                    nc.tensor.matmul(
                        ps[:],
                        wslice(nm, k, m * 128, (m + 1) * 128),
                        xsl(k, n),
                        start=(k == 0), stop=(k == KC - 1))
                nc.vector.tensor_copy(dst[:, n * 512:(n + 1) * 512], ps[:])

            def emit_va_group(sc):
                t = va[sc]
                n, r = divmod(sc, 4)
                tones = t[:].rearrange("p (h x) -> p h x", x=65)[:, :, 64:65]
                nc.vector.tensor_copy(
                    tones, onesc[:].rearrange("p (h x) -> p h x", x=1))
                ps = psum_p.tile([128, CPB], f32, tag="psp", name="psv")
                for k in range(KC):
                    nc.tensor.matmul(
                        ps[:],
                        xsl(k, n, r * 128, (r + 1) * 128),
                        wslice("wv", k, 0, CPB),
                        start=(k == 0), stop=(k == KC - 1))
                tv = t[:].rearrange("p (h x) -> p h x", x=65)[:, :, 0:64]
                pv = ps[:].rearrange("p (h d) -> p h d", d=64)
                nc.vector.tensor_copy(tv, pv)

            def emit_transpose(sc):
                for half in range(2):
                    pt = psum_p.tile([128, 128], bf16, tag="psp", name="pt")
                    nc.tensor.transpose(
                        pt[:], on_tiles[sc][:, half * 128:(half + 1) * 128],
                        ident[:])
                    nc.vector.tensor_copy(
                        ot_tiles[half][:, sc * 128:(sc + 1) * 128], pt[:])

            def emit_y_proj(sc):
                ys = ysb_pool.tile([128, C], bf16, tag="ysb")
                pss2 = [psum_p.tile([128, 512], f32, tag="psp", name="psy")
                        for _ in range(2)]
                for m in range(2):  # m outer: reuse stationary across n2
                    for n2 in range(2):
                        nc.tensor.matmul(
                            pss2[n2][:],
                            ot_tiles[m][:, sc * 128:(sc + 1) * 128],
                            wp_sb[:, m * C + n2 * 512:m * C + (n2 + 1) * 512],
                            start=(m == 0), stop=(m == 1))
                nc.vector.tensor_copy(ys[:, 0:512], pss2[0][:])
                nc.vector.tensor_copy(ys[:, 512:1024], pss2[1][:])
                nc.sync.dma_start(y[sc * 128:(sc + 1) * 128, :], ys[:])

            # filler queue: PE work emitted between EXP-dependent attention
            # ops so the PE FIFO never stalls waiting on the Scalar engine.
            # Entries carry a (kind, key) tag so dependency deadlines can
            # force-drain them before the attention work that consumes them.
            fillers = deque()  # (kind, key, avail_qv, emit_fn)

            def take_filler(cur_qv, n=1):
                # pop the first filler whose availability window has opened
                # (tr/y fillers are held back so the late, filler-hungry
                # views still have PE work to interleave)
                for _ in range(n):
                    for idx, (kind, key, avail, fn) in enumerate(fillers):
                        if avail <= cur_qv:
                            del fillers[idx]
                            fn()
                            break
                    else:
                        return

            def drain_fillers(pred):
                keep = deque()
                while fillers:
                    kind, key, avail, fn = fillers.popleft()
                    if pred(kind, key):
                        fn()
                    else:
                        keep.append((kind, key, avail, fn))
                fillers.extend(keep)

            def push_panel_fillers(n):
                for m in range(2):
                    for p in ("q", "k"):
                        fillers.append(
                            ("qk", n, 0,
                             lambda p=p, m=m, n=n: emit_qk_proj_group(p, m, n)))
                for sc in range(4 * n, 4 * n + 4):
                    fillers.append(
                        ("va", sc, 0, lambda sc=sc: emit_va_group(sc)))

            def emit_qk_phase(m, qv):
                kT = qk_tiles[("k", m)]
                qT = qk_tiles[("q", m)]
                qs = slice(qv * 256, (qv + 1) * 256)
                kvs = _allowed(qv)
                ets = []
                for i, kv in enumerate(kvs):
                    pss = psum_s.tile([128, 1024], f32, tag="pss", name="pss")
                    for j in range(2):
                        kc = 2 * kv + j
                        for h in range(2):
                            nc.tensor.matmul(
                                pss[:, (2 * h + j) * 256:
                                    (2 * h + j + 1) * 256],
                                kT[64 * h:64 * (h + 1),
                                   kc * 128:(kc + 1) * 128],
                                qT[64 * h:64 * (h + 1), qs],
                                start=True, stop=True)
                    et = exp_pool.tile([128, 1024], bf16, tag="exp", bufs=20)
                    nc.scalar.activation(et[:], pss[:], EXP,
                                         scale=float(SCALE))
                    ets.append((kv, et))
                    if i >= 1:
                        take_filler(qv, 1)
                if len(kvs) <= 3:
                    take_filler(qv, 1)
                return ets

            def emit_pv_phase(m, qv, ets, qcs=(0, 1)):
                # PV accumulation groups per (h, q-chunk); the ones column of
                # va lands the softmax denominator in PSUM column 64
                last = len(ets) - 1
                rp = small_pool.tile([128, 4], f32, tag="rp", bufs=4,
                                     name="rp")
                for h in range(2):
                    hh = 2 * m + h
                    for qc in qcs:
                        g = 2 * h + qc
                        pg = psum_o.tile([128, 65], f32, tag="pso",
                                         name=f"pg{g}")
                        for i, (kv, et) in enumerate(ets):
                            for j in range(2):
                                kc = 2 * kv + j
                                nc.tensor.matmul(
                                    pg[:],
                                    et[:, (2 * h + j) * 256 + qc * 128:
                                       (2 * h + j) * 256 + qc * 128 + 128],
                                    va[kc][:, hh * 65:(hh + 1) * 65],
                                    start=(i == 0 and j == 0),
                                    stop=(i == last and j == 1))
                        sc = qv * 2 + qc
                        nc.vector.reciprocal(rp[:, g:g + 1], pg[:, 64:65])
                        nc.vector.tensor_scalar_mul(
                            on_tiles[sc][:, hh * 64:(hh + 1) * 64],
                            pg[:, 0:64],
                            rp[:, g:g + 1])

            # ---- main emission: panel 0 directly, then attention with
            # later panels' projections + transposes/y-projs as PE fillers.
            # Per view: both head-pairs' QK phases back-to-back (m1's QK
            # covers m0's EXP latency), then both PV phases. ----
            for m in range(2):
                for p in ("q", "k"):
                    emit_qk_proj_group(p, m, 0)
            for sc in range(4):
                emit_va_group(sc)
            push_panel_fillers(1)
            for n in range(NS):
                for qv in (2 * n, 2 * n + 1):
                    # hard deadlines: this view's q/k panels + va chunks
                    drain_fillers(lambda kind, key, qv=qv:
                                  (kind == "qk" and key <= qv // 2) or
                                  (kind == "va" and key <= 2 * qv + 1))
                    ets0 = emit_qk_phase(0, qv)
                    ets1 = emit_qk_phase(1, qv)
                    emit_pv_phase(0, qv, ets0)
                    emit_pv_phase(1, qv, ets1)
                    for sc in (2 * qv, 2 * qv + 1):
                        avail = min(7, sc // 2 + 3)
                        fillers.append(
                            ("tr", sc, avail, lambda sc=sc: emit_transpose(sc)))
                        fillers.append(
                            ("y", sc, avail, lambda sc=sc: emit_y_proj(sc)))
                if n + 2 <= NS - 1:
                    push_panel_fillers(n + 2)
            import os
            if os.environ.get("DBG_FILLERS"):
                sys.stderr.write(
                    f"leftover fillers at tail: {[(k, key) for k, key, _, _ in fillers]}\n")
            while fillers:
                fillers.popleft()[3]()

    nc.compile()
    return nc


def _get_compiled():
    if "nc" not in _compiled:
        _compiled["nc"] = build()
    return _compiled["nc"]


def make_in_maps(x, Wq, Wk, Wv, Wp):
    bf = ml_dtypes.bfloat16
    xf = np.asarray(x, np.float32).reshape(B, S, C)

    def pack_w(wT):  # [C, CPB] -> [128, KC*CPB]
        return np.ascontiguousarray(
            wT.reshape(KC, 128, CPB).transpose(1, 0, 2).reshape(128, KC * CPB)
        ).astype(bf)

    in_maps = []
    for c in range(N_CORES):
        b, g = divmod(c, HPC)
        hs = slice(g * CPB, (g + 1) * CPB)
        xT = xf[b].T  # [C, S]
        xPm = np.ascontiguousarray(
            xT.reshape(KC, 128, NS, 512)
            .transpose(1, 0, 2, 3).reshape(128, KC * S)).astype(bf)
        wpT = np.asarray(Wp, np.float32)[:, hs].T  # [CPB, C]
        wpm = np.ascontiguousarray(
            wpT.reshape(2, 128, C).transpose(1, 0, 2).reshape(128, 2 * C)
        ).astype(bf)
        in_maps.append({
            "xP": xPm,
            "wqP": pack_w(np.asarray(Wq, np.float32)[hs].T),
            "wkP": pack_w(np.asarray(Wk, np.float32)[hs].T),
            "wvP": pack_w(np.asarray(Wv, np.float32)[hs].T),
            "wpP": wpm,
        })
    return in_maps


def kernel(x, Wq, Wk, Wv, Wp, bp, _trace=False, _tmpdir=None):
    global LAST_RESULTS
    from concourse import bass_utils

    nc = _get_compiled()
    in_maps = make_in_maps(x, Wq, Wk, Wv, Wp)
    kwargs = {}
    if _trace:
        kwargs = {"trace": True, "tmpdir": _tmpdir}
    res = bass_utils.run_bass_kernel_spmd(
        nc, in_maps, core_ids=list(range(N_CORES)), **kwargs)
    LAST_RESULTS = res
    yout = np.zeros((B, S, C), np.float32)
    for c in range(N_CORES):
        yout[c // HPC] += res.results[c]["y"].astype(np.float32)
    yout += np.asarray(bp, np.float32).reshape(1, 1, C)
    return yout.reshape(B, V, L, C)


# revision 24
# speedup vs baseline: 1.0427x; 1.0220x over previous
"""Block-sparse (view-causal) multi-head attention on 8 TRN2 NeuronCores.

Full inputs in, full output out. Sharding: data-parallel over batch (B=2),
tensor-parallel over heads (16 heads -> 4 per core). Each core computes its
4 heads' attention + its slice of the output projection; the host sums the
4 head-group partial projections per batch (the tensor-parallel reduce).

Device-side pipeline (single fused emission stream, phases interleaved so
the Scalar engine's EXP work overlaps PE work instead of serializing):
  - inputs DMA'd in 512-token panels so projections start early
  - q/k kept transposed (qT/kT [dh, tokens]) so score matmuls need no
    transposes; V is augmented with a ones column so each PV matmul
    accumulates both sum(exp*v) and the softmax denominator in PSUM
  - attention emitted per query view; "filler" PE work (later panels'
    projections, output projections) is interleaved between the
    EXP-dependent attention ops so the PE FIFO never stalls on Scalar
  - per-view output projection + DMA so the 4MB output streams out
    during the attention phase instead of after it
All matmul operands are bf16 (PSUM accumulation in fp32).
"""

import sys
from collections import deque

if "/opt/trn_rl_repo" not in sys.path:
    sys.path.insert(0, "/opt/trn_rl_repo")

import numpy as np
import ml_dtypes

B, V, L, C, H = 2, 8, 256, 1024, 16
S = V * L                # 2048 tokens
DH = C // H              # 64
HPC = 4                  # heads per core
CPB = HPC * DH           # 256 channel block per core
N_CORES = 8
SCALE = DH ** -0.5       # 1/8, folded into the exp activation
KC = C // 128            # 8 contraction chunks for the projections
NS = 4                   # 512-token panels
SC = S // 128            # 16 sequence chunks

_compiled = {}
LAST_RESULTS = None
# legacy flags kept for test-harness compatibility
PACK_QK = True
SAFE_RECIP = False
SPLIT_ACT = False


def _allowed(qv):
    """View-level mask row: views 0/1 cross-attend only; views >=2 block-causal."""
    if qv == 0:
        return [1]
    if qv == 1:
        return [0]
    return list(range(qv + 1))


def build():
    import concourse.tile as tile
    from concourse import bacc, mybir
    from concourse.masks import make_identity

    f32 = mybir.dt.float32
    bf16 = mybir.dt.bfloat16
    f8 = mybir.dt.float8e4
    EXP = mybir.ActivationFunctionType.Exp

    nc = bacc.Bacc("TRN2", target_bir_lowering=False, debug=False,
                   num_devices=N_CORES)
    xP = nc.dram_tensor("xP", [128, KC * S], bf16, kind="ExternalInput").ap()
    wqP = nc.dram_tensor("wqP", [128, KC * CPB], bf16, kind="ExternalInput").ap()
    wkP = nc.dram_tensor("wkP", [128, KC * CPB], bf16, kind="ExternalInput").ap()
    wvP = nc.dram_tensor("wvP", [128, KC * CPB], bf16, kind="ExternalInput").ap()
    wpP = nc.dram_tensor("wpP", [128, 2 * C], bf16, kind="ExternalInput").ap()
    y = nc.dram_tensor("y", [S, C], bf16, kind="ExternalOutput").ap()

    HW = KC * CPB // 2   # half-weight free size

    with tile.TileContext(nc) as tc:
        with (
            tc.tile_pool(name="wts", bufs=1) as w_pool,
            tc.tile_pool(name="xt", bufs=1) as xt_pool,
            tc.tile_pool(name="qk", bufs=1) as qk_pool,
            tc.tile_pool(name="va", bufs=1) as va_pool,
            tc.tile_pool(name="ot", bufs=1) as ot_pool,
            tc.tile_pool(name="on", bufs=1) as on_pool,
            tc.tile_pool(name="exp", bufs=1) as exp_pool,
            tc.tile_pool(name="small", bufs=1) as small_pool,
            tc.tile_pool(name="ysb", bufs=3) as ysb_pool,
            tc.tile_pool(name="psp", bufs=2, space="PSUM") as psum_p,
            tc.tile_pool(name="pss", bufs=2, space="PSUM") as psum_s,
            tc.tile_pool(name="pso", bufs=2, space="PSUM") as psum_o,
        ):
            # ---- weights (in halves for earlier readiness) + x panels ----
            wh = {}
            for nm, dram in (("wq", wqP), ("wk", wkP), ("wv", wvP)):
                for half in range(2):
                    t = w_pool.tile([128, HW], bf16, tag=f"{nm}{half}")
                    wh[(nm, half)] = t
            wp_sb = w_pool.tile([128, 2 * C], bf16, tag="wp")
            def wslice(nm, k, lo, hi):
                # columns [lo, hi) of contraction-chunk k of weight nm
                half, kk = divmod(k, KC // 2)
                return wh[(nm, half)][:, kk * CPB + lo:kk * CPB + hi]

            # trn2 has exactly two HW-DGE rings (sync + scalar), each FIFO:
            # split the startup-critical transfers across both so panel 0
            # and wq/wk arrive as early as possible.
            xts = {}  # (k, n) -> (tile, col offset)

            def xsl(k, n, lo=0, hi=512):
                t, base = xts[(k, n)]
                return t[:, base + lo:base + hi]

            def dma_x(k, n, eng, width=512):
                t = xt_pool.tile([128, width], bf16, tag=f"xt{width}",
                                 bufs=(16 if width == 512 else 8),
                                 name=f"xt{k}_{n}")
                eng.dma_start(
                    t[:], xP[:, (k * NS + n) * 512:(k * NS + n) * 512 + width])
                for w in range(width // 512):
                    xts[(k, n + w)] = (t, w * 512)

            # ring A (sync) / ring B (scalar), interleaved by arrival need
            nc.scalar.dma_start(wh[("wq", 0)][:], wqP[:, 0:HW])
            for k in range(4):
                dma_x(k, 0, nc.sync)
            nc.scalar.dma_start(wh[("wq", 1)][:], wqP[:, HW:2 * HW])
            for k in range(4, KC):
                dma_x(k, 0, nc.scalar)
            nc.sync.dma_start(wh[("wk", 0)][:], wkP[:, 0:HW])
            nc.sync.dma_start(wh[("wk", 1)][:], wkP[:, HW:2 * HW])
            nc.scalar.dma_start(wh[("wv", 0)][:], wvP[:, 0:HW])
            nc.scalar.dma_start(wh[("wv", 1)][:], wvP[:, HW:2 * HW])
            for k in range(KC):  # panel 1 split across both rings
                dma_x(k, 1, nc.sync if k % 2 == 0 else nc.scalar)
            nc.scalar.dma_start(wp_sb[:], wpP[:, :])
            for k in range(KC):  # panels 2+3 merged into 256KB transfers
                dma_x(k, 2, nc.sync if k % 2 == 0 else nc.scalar, width=1024)

            # ---- constants + PE clock warmup while input DMAs run ----
            onesc = small_pool.tile([128, HPC], bf16, tag="onesc")
            nc.vector.memset(onesc[:], 1.0)
            junkw = small_pool.tile([128, 128], bf16, tag="junkw")
            nc.vector.memset(junkw[:], 0.5)
            junkm = small_pool.tile([128, 256], bf16, tag="junkm")
            nc.vector.memset(junkm[:], 0.5)
            for _ in range(10):
                wps = psum_p.tile([128, 512], f32, tag="psp", name="warm")
                nc.tensor.matmul(wps[:, 0:256], junkw[:], junkm[:],
                                 start=True, stop=True)
            # identity for the PE-mode transposes; first needed mid-kernel
            ident = small_pool.tile([128, 128], bf16, tag="ident")
            make_identity(nc, ident[:])

            # ---- persistent activation tiles ----
            qk_tiles = {}
            for p in ("q", "k"):
                for m in range(2):
                    qk_tiles[(p, m)] = qk_pool.tile(
                        [128, S], bf16, tag=f"{p}T{m}", name=f"{p}T{m}")
            va = [va_pool.tile([128, HPC * 65], bf16, tag="va", bufs=SC,
                               name=f"va{sc}") for sc in range(SC)]
            on_tiles = [on_pool.tile([128, CPB], bf16, tag="on", bufs=SC,
                                     name=f"on{sc}") for sc in range(SC)]
            ot_tiles = [ot_pool.tile([128, S], bf16, tag=f"ot{m}",
                                     name=f"oT{m}") for m in range(2)]

            # ---- emitters ----
            def emit_qk_proj_group(p, m, n):
                nm = "wq" if p == "q" else "wk"
                dst = qk_tiles[(p, m)]
                ps = psum_p.tile([128, 512], f32, tag="psp", name="psproj")
                for k in range(KC):
                    nc.tensor.matmul(
                        ps[:],
                        wslice(nm, k, m * 128, (m + 1) * 128),
                        xsl(k, n),
                        start=(k == 0), stop=(k == KC - 1))
                nc.vector.tensor_copy(dst[:, n * 512:(n + 1) * 512], ps[:])

            def emit_va_group(sc):
                t = va[sc]
                n, r = divmod(sc, 4)
                tones = t[:].rearrange("p (h x) -> p h x", x=65)[:, :, 64:65]
                nc.vector.tensor_copy(
                    tones, onesc[:].rearrange("p (h x) -> p h x", x=1))
                ps = psum_p.tile([128, CPB], f32, tag="psp", name="psv")
                for k in range(KC):
                    nc.tensor.matmul(
                        ps[:],
                        xsl(k, n, r * 128, (r + 1) * 128),
                        wslice("wv", k, 0, CPB),
                        start=(k == 0), stop=(k == KC - 1))
                tv = t[:].rearrange("p (h x) -> p h x", x=65)[:, :, 0:64]
                pv = ps[:].rearrange("p (h d) -> p h d", d=64)
                nc.vector.tensor_copy(tv, pv)

            def emit_transpose(sc):
                for half in range(2):
                    pt = psum_p.tile([128, 128], bf16, tag="psp", name="pt")
                    nc.tensor.transpose(
                        pt[:], on_tiles[sc][:, half * 128:(half + 1) * 128],
                        ident[:])
                    nc.vector.tensor_copy(
                        ot_tiles[half][:, sc * 128:(sc + 1) * 128], pt[:])

            def emit_y_proj(sc):
                ys = ysb_pool.tile([128, C], bf16, tag="ysb")
                pss2 = [psum_p.tile([128, 512], f32, tag="psp", name="psy")
                        for _ in range(2)]
                for m in range(2):  # m outer: reuse stationary across n2
                    for n2 in range(2):
                        nc.tensor.matmul(
                            pss2[n2][:],
                            ot_tiles[m][:, sc * 128:(sc + 1) * 128],
                            wp_sb[:, m * C + n2 * 512:m * C + (n2 + 1) * 512],
                            start=(m == 0), stop=(m == 1))
                nc.vector.tensor_copy(ys[:, 0:512], pss2[0][:])
                nc.vector.tensor_copy(ys[:, 512:1024], pss2[1][:])
                nc.sync.dma_start(y[sc * 128:(sc + 1) * 128, :], ys[:])

            # filler queue: PE work emitted between EXP-dependent attention
            # ops so the PE FIFO never stalls waiting on the Scalar engine.
            # Entries carry a (kind, key) tag so dependency deadlines can
            # force-drain them before the attention work that consumes them.
            fillers = deque()  # (kind, key, avail_qv, emit_fn)

            def take_filler(cur_qv, n=1):
                # pop the first filler whose availability window has opened
                # (tr/y fillers are held back so the late, filler-hungry
                # views still have PE work to interleave)
                for _ in range(n):
                    for idx, (kind, key, avail, fn) in enumerate(fillers):
                        if avail <= cur_qv:
                            del fillers[idx]
                            fn()
                            break
                    else:
                        return

            def drain_fillers(pred):
                keep = deque()
                while fillers:
                    kind, key, avail, fn = fillers.popleft()
                    if pred(kind, key):
                        fn()
                    else:
                        keep.append((kind, key, avail, fn))
                fillers.extend(keep)

            def push_panel_fillers(n):
                for m in range(2):
                    for p in ("q", "k"):
                        fillers.append(
                            ("qk", n, 0,
                             lambda p=p, m=m, n=n: emit_qk_proj_group(p, m, n)))
                for sc in range(4 * n, 4 * n + 4):
                    fillers.append(
                        ("va", sc, 0, lambda sc=sc: emit_va_group(sc)))

            def emit_qk_phase(m, qv):
                kT = qk_tiles[("k", m)]
                qT = qk_tiles[("q", m)]
                qs = slice(qv * 256, (qv + 1) * 256)
                kvs = _allowed(qv)
                ets = []
                for i, kv in enumerate(kvs):
                    pss = psum_s.tile([128, 1024], f32, tag="pss", name="pss")
                    for j in range(2):
                        kc = 2 * kv + j
                        for h in range(2):
                            nc.tensor.matmul(
                                pss[:, (2 * h + j) * 256:
                                    (2 * h + j + 1) * 256],
                                kT[64 * h:64 * (h + 1),
                                   kc * 128:(kc + 1) * 128],
                                qT[64 * h:64 * (h + 1), qs],
                                start=True, stop=True)
                    et = exp_pool.tile([128, 1024], bf16, tag="exp", bufs=20)
                    nc.scalar.activation(et[:], pss[:], EXP,
                                         scale=float(SCALE))
                    ets.append((kv, et))
                    if i >= 1:
                        take_filler(qv, 1)
                if len(kvs) <= 3:
                    take_filler(qv, 1)
                return ets

            def emit_pv_phase(m, qv, ets, qcs=(0, 1)):
                # PV accumulation groups per (h, q-chunk); the ones column of
                # va lands the softmax denominator in PSUM column 64
                last = len(ets) - 1
                rp = small_pool.tile([128, 4], f32, tag="rp", bufs=4,
                                     name="rp")
                for h in range(2):
                    hh = 2 * m + h
                    for qc in qcs:
                        g = 2 * h + qc
                        pg = psum_o.tile([128, 65], f32, tag="pso",
                                         name=f"pg{g}")
                        for i, (kv, et) in enumerate(ets):
                            for j in range(2):
                                kc = 2 * kv + j
                                nc.tensor.matmul(
                                    pg[:],
                                    et[:, (2 * h + j) * 256 + qc * 128:
                                       (2 * h + j) * 256 + qc * 128 + 128],
                                    va[kc][:, hh * 65:(hh + 1) * 65],
                                    start=(i == 0 and j == 0),
                                    stop=(i == last and j == 1))
                        sc = qv * 2 + qc
                        nc.vector.reciprocal(rp[:, g:g + 1], pg[:, 64:65])
                        nc.vector.tensor_scalar_mul(
                            on_tiles[sc][:, hh * 64:(hh + 1) * 64],
                            pg[:, 0:64],
                            rp[:, g:g + 1])

            # ---- main emission: panel 0 directly, then attention with
            # later panels' projections + transposes/y-projs as PE fillers.
            # Per view: both head-pairs' QK phases back-to-back (m1's QK
            # covers m0's EXP latency), then both PV phases. ----
            for m in range(2):
                for p in ("q", "k"):
                    emit_qk_proj_group(p, m, 0)
            for sc in range(4):
                emit_va_group(sc)
            push_panel_fillers(1)
            for n in range(NS):
                for qv in (2 * n, 2 * n + 1):
                    # hard deadlines: this view's q/k panels + va chunks
                    drain_fillers(lambda kind, key, qv=qv:
                                  (kind == "qk" and key <= qv // 2) or
                                  (kind == "va" and key <= 2 * qv + 1))
                    ets0 = emit_qk_phase(0, qv)
                    ets1 = emit_qk_phase(1, qv)
                    emit_pv_phase(0, qv, ets0)
                    emit_pv_phase(1, qv, ets1)
                    for sc in (2 * qv, 2 * qv + 1):
                        avail = min(7, sc // 2 + 3)
                        fillers.append(
                            ("tr", sc, avail, lambda sc=sc: emit_transpose(sc)))
                        fillers.append(
                            ("y", sc, avail, lambda sc=sc: emit_y_proj(sc)))
                if n + 2 <= NS - 1:
                    push_panel_fillers(n + 2)
            import os
            if os.environ.get("DBG_FILLERS"):
                sys.stderr.write(
                    f"leftover fillers at tail: {[(k, key) for k, key, _, _ in fillers]}\n")
            while fillers:
                fillers.popleft()[3]()

    nc.compile()
    return nc


def _get_compiled():
    if "nc" not in _compiled:
        _compiled["nc"] = build()
    return _compiled["nc"]


def make_in_maps(x, Wq, Wk, Wv, Wp):
    bf = ml_dtypes.bfloat16
    xf = np.asarray(x, np.float32).reshape(B, S, C)

    def pack_w(wT):  # [C, CPB] -> [128, KC*CPB]
        return np.ascontiguousarray(
            wT.reshape(KC, 128, CPB).transpose(1, 0, 2).reshape(128, KC * CPB)
        ).astype(bf)

    in_maps = []
    for c in range(N_CORES):
        b, g = divmod(c, HPC)
        hs = slice(g * CPB, (g + 1) * CPB)
        xT = xf[b].T  # [C, S]
        xPm = np.ascontiguousarray(
            xT.reshape(KC, 128, NS, 512)
            .transpose(1, 0, 2, 3).reshape(128, KC * S)).astype(bf)
        wpT = np.asarray(Wp, np.float32)[:, hs].T  # [CPB, C]
        wpm = np.ascontiguousarray(
            wpT.reshape(2, 128, C).transpose(1, 0, 2).reshape(128, 2 * C)
        ).astype(bf)
        in_maps.append({
            "xP": xPm,
            "wqP": pack_w(np.asarray(Wq, np.float32)[hs].T),
            "wkP": pack_w(np.asarray(Wk, np.float32)[hs].T),
            "wvP": pack_w(np.asarray(Wv, np.float32)[hs].T),
            "wpP": wpm,
        })
    return in_maps


def kernel(x, Wq, Wk, Wv, Wp, bp, _trace=False, _tmpdir=None):
    global LAST_RESULTS
    from concourse import bass_utils

    nc = _get_compiled()
    in_maps = make_in_maps(x, Wq, Wk, Wv, Wp)
    kwargs = {}
    if _trace:
        kwargs = {"trace": True, "tmpdir": _tmpdir}
    res = bass_utils.run_bass_kernel_spmd(
        nc, in_maps, core_ids=list(range(N_CORES)), **kwargs)
    LAST_RESULTS = res
    yout = np.zeros((B, S, C), np.float32)
    for c in range(N_CORES):
        yout[c // HPC] += res.results[c]["y"].astype(np.float32)
    yout += np.asarray(bp, np.float32).reshape(1, 1, C)
    return yout.reshape(B, V, L, C)
